# revision 1
# baseline (speedup 1.0000x reference)
"""Trainium2 Bass kernel for the MERITS_T patient model (B=1024 data-parallel over 8 cores).

Mathematical simplification of the reference (verified to ~4e-7 rel err):
  - E_de = _mha(drug_mem, e0, e0) softmaxes over a single key, so its output is
    e0 @ m2_wv @ m2_wo broadcast over all 145 query rows -> the three GATs, the
    graph MHA and drug_mem never reach the output (dead code).
  - e0 = E_en[:, 0] only needs query row 0 of the m1 attention, i.e. only the
    first visit of `med`.
  - Per-head attention is refactored as u_h = mr0 @ (wq_h wk_h^T / sqrt(dh)),
    s_j = u_h . patient_j, r = sum_h (softmax-weighted patient avg) @ (wv_h wo_h m2_wv m2_wo).
  - final reshape tiles r 145x, so relu(final) @ out_w1 = relu(r) @ sum_m out_w1[m].
    The 43MB sum over m is sharded 8 ways and AllReduced on-device.

Per-core work (128 patients): static MLP over lab, glu/med encoders, one-query
attention over 25 visits, final MLP [64]->[1160]->[145].
"""

import numpy as np

import concourse.bass as bass
import concourse.mybir as mybir
from concourse.bass_utils import run_bass_kernel_spmd
from concourse.tile import TileContext

F32 = mybir.dt.float32
AF = mybir.ActivationFunctionType
ALU = mybir.AluOpType
AX = mybir.AxisListType

def split_multi_waits(nc):
    """The walrus on this image encodes at most ONE sync wait per TPB
    instruction ("Too many sync wait commands" otherwise). Hoist excess waits
    onto standalone InstEventSemaphore ops on the same engine, immediately
    before the instruction — the same mechanism Tile's barriers use."""
    wid = 0
    for f in nc.m.functions:
        for bb in f.blocks:
            out = []
            for ins in bb.instructions:
                si = ins.sync_info
                if si is not None and si.on_wait and len(si.on_wait) > 1:
                    waits = list(si.on_wait)
                    for w in waits[:-1]:
                        wid += 1
                        out.append(mybir.InstEventSemaphore(
                            name=f"Wsplit-{wid}", engine=ins.engine,
                            ins=[], outs=[],
                            sync_info=mybir.SyncInfo(on_wait=[w], on_update=[])))
                    si.on_wait = waits[-1:]
                out.append(ins)
            bb.instructions = out
    return wid


B, T, MED, LAB, GLU, D, H = 1024, 25, 145, 1956, 16, 64, 32
NC_CORES = 8
BC = B // NC_CORES  # 128 patients per core
NH, DH = 4, 16
HID = MED * D // 8  # 1160
MBLK = 19  # ceil(145/8) out_w1 blocks per core (zero-padded)


def build_bass(split_waits=True):
    nc = bass.Bass()

    # ---- I/O declarations (per-core shapes) ----
    def inp(name, shape):
        return nc.dram_tensor(name, list(shape), F32, kind="ExternalInput")

    lab_d = inp("lab", (BC, LAB + 1))  # ones column appended (bias fold)
    glu_d = inp("glu", (BC, T, GLU))
    tf_d = inp("tf", (BC, T, GLU))
    med_d = inp("med", (BC, T, MED))
    w1s_d = inp("w1shard", (MBLK, D, HID))
    sllw1_d = inp("sll_w1", (LAB + 1, D))  # bias row appended
    sllw2_d = inp("sll_w2", (D + 1, H))    # bias row appended
    gluw_d = inp("glu_w", (2 * GLU, H))
    glub_d = inp("glu_b", (1, H))
    glug_d = inp("glu_gate", (1, H))
    medw_d = inp("med_w", (MED + 1, D))    # bias row appended
    medg_d = inp("med_gate", (1, D))
    wq_d = inp("m1_wq", (D, D))
    wk_d = inp("m1_wk", (D, D))
    wv_d = inp("m1_wv", (D, D))
    wo_d = inp("m1_wo", (D, D))
    m2wv_d = inp("m2_wv", (D, D))
    m2wo_d = inp("m2_wo", (D, D))
    outb1_d = inp("out_b1", (1, HID))
    outw2_d = inp("out_w2", (HID + 1, MED))  # bias row appended
    out_d = nc.dram_tensor("out", [BC, MED], F32, kind="ExternalOutput")

    # Inline constants: per-head partition masks and the glu block-diag mask
    hm = np.zeros((D, NH), np.float32)
    for h in range(NH):
        hm[h * DH:(h + 1) * DH, h] = 1.0
    hmask_d = nc.inline_tensor(hm, name="head_mask")
    bdm = np.zeros((128, 8 * H), np.float32)
    for jl in range(8):
        bdm[jl * GLU:(jl + 1) * GLU, jl * H:(jl + 1) * H] = 1.0
    bdmask_d = nc.inline_tensor(bdm, name="bd_mask")
    ident_d = nc.inline_tensor(np.eye(128, dtype=np.float32), name="ident128")

    # Internal DRAM for the W1s AllReduce (kept in the [128, 580] layout the
    # on-chip reduce produces; the [64, 1160] regather is a linear DRAM view)
    cc_in = nc.dram_tensor("cc_in", [128, 580], F32)
    cc_out = nc.dram_tensor("cc_out", [128, 580], F32, addr_space="Shared")

    with TileContext(nc) as tc, \
            tc.tile_pool(name="consts", bufs=1) as cp, \
            tc.tile_pool(name="ps", bufs=2, space="PSUM") as ps, \
            tc.tile_pool(name="psg", bufs=1, space="PSUM") as psg:

        dma = nc.sync.dma_start

        # ================= W1s shard sum + AllReduce (long latency, start early) ====
        # shard viewed as [(f h)=128 partitions, m=19, i=580]
        w1v = w1s_d[:].rearrange("m f (h i) -> (f h) m i", h=2)
        w1raw = cp.tile([128, MBLK, 580], F32, tag="w1raw")
        for q in range(4):
            dma(out=w1raw[:, :, q * 145:(q + 1) * 145],
                in_=w1v[:, :, q * 145:(q + 1) * 145])
        w1red = cp.tile([128, 580], F32, tag="w1red")
        for q in range(4):  # one reduce per DMA chunk (ISA sync-wait limit)
            nc.vector.tensor_reduce(
                out=w1red[:, q * 145:(q + 1) * 145],
                in_=w1raw[:, :, q * 145:(q + 1) * 145].rearrange("p m i -> p i m"),
                axis=AX.X, op=ALU.add)
        dma(out=cc_in[:], in_=w1red[:])
        nc.gpsimd.collective_compute(
            "AllReduce", ALU.add, replica_groups=[list(range(NC_CORES))],
            ins=[cc_in[:]], outs=[cc_out[:]])
        w1s_sb = cp.tile([D + 1, HID], F32, tag="w1s_sb")
        dma(out=w1s_sb[0:D, :], in_=cc_out[:].rearrange("(f h) i -> f (h i)", h=2))
        dma(out=w1s_sb[D:D + 1, :], in_=outb1_d[:])

        # ================= constants / weights =====================================
        ident = cp.tile([128, 128], F32, tag="ident")
        dma(out=ident, in_=ident_d[:])

        # sll_w1 tiles [128, 16, 64]; host already appended the bias row
        w1sb = cp.tile([128, 16, D], F32, tag="w1sb")
        dma(out=w1sb[:, 0:15, :], in_=sllw1_d[0:1920, :].rearrange("(t k) d -> k t d", k=128))
        dma(out=w1sb[0:37, 15, :], in_=sllw1_d[1920:1957, :])
        w2sb = cp.tile([D + 1, H], F32, tag="w2sb")
        dma(out=w2sb, in_=sllw2_d[:])
        gw_g3 = cp.tile([GLU, H], F32, tag="gw_g3")
        dma(out=gw_g3, in_=gluw_d[0:GLU, :])
        gw_t3 = cp.tile([GLU, H], F32, tag="gw_t3")
        dma(out=gw_t3, in_=gluw_d[GLU:2 * GLU, :])
        # block-diagonal glu weights: one broadcast DMA replicates glu_w[rows]
        # into every (jl, jl') block of [128, 256]; an inline 0/1 mask then
        # zeroes the off-diagonal blocks. One K=128 matmul projects 8 visits.
        bdmask = cp.tile([128, 8 * H], F32, tag="bdmask")
        dma(out=bdmask, in_=bdmask_d[:])

        def build_wbd(row0, tag):
            rep = cp.tile([128, H], F32, tag=tag + "_rep")
            dma(out=rep,
                in_=gluw_d[row0:row0 + GLU, :].unsqueeze(0).broadcast_to((8, GLU, H)))
            wbd = cp.tile([128, 8, H], F32, tag=tag)
            nc.vector.tensor_mul(wbd,
                                 rep[:].unsqueeze(1).broadcast_to((128, 8, H)),
                                 bdmask[:].rearrange("p (j o) -> p j o", j=8))
            return wbd

        wbd_g = build_wbd(0, "wbd_g")
        wbd_t = build_wbd(GLU, "wbd_t")
        gbb = cp.tile([128, H], F32, tag="gbb")
        dma(out=gbb, in_=glub_d[:].broadcast_to((128, H)))
        ggb = cp.tile([128, H], F32, tag="ggb")
        dma(out=ggb, in_=glug_d[:].broadcast_to((128, H)))
        mwsb = cp.tile([128, D], F32, tag="mwsb")
        dma(out=mwsb, in_=medw_d[0:128, :])
        mw2sb = cp.tile([18, D], F32, tag="mw2sb")
        dma(out=mw2sb, in_=medw_d[128:146, :])
        mgb = cp.tile([128, D], F32, tag="mgb")
        dma(out=mgb, in_=medg_d[:].broadcast_to((128, D)))
        ow2sb = cp.tile([128, 10, MED], F32, tag="ow2sb")
        dma(out=ow2sb[:, 0:9, :], in_=outw2_d[0:1152, :].rearrange("(t k) n -> k t n", k=128))
        dma(out=ow2sb[0:9, 9, :], in_=outw2_d[1152:1161, :])

        wq_sb = cp.tile([D, D], F32, tag="wq_sb")
        dma(out=wq_sb, in_=wq_d[:])
        wk_sb = cp.tile([D, D], F32, tag="wk_sb")
        dma(out=wk_sb, in_=wk_d[:])
        wv_sb = cp.tile([D, D], F32, tag="wv_sb")
        dma(out=wv_sb, in_=wv_d[:])
        wo_sb = cp.tile([D, D], F32, tag="wo_sb")
        dma(out=wo_sb, in_=wo_d[:])
        m2wv_sb = cp.tile([D, D], F32, tag="m2wv_sb")
        dma(out=m2wv_sb, in_=m2wv_d[:])
        m2wo_sb = cp.tile([D, D], F32, tag="m2wo_sb")
        dma(out=m2wo_sb, in_=m2wo_d[:])

        # ---- weight prep on PE: wqT/wkT/wvT/m2wvT, A_h, M_hT, Wvo2, MW_h ----
        def transpose_to_sbuf(src_ap, rows, cols, sb_tile, copy_engine=None):
            pt = ps.tile([cols, rows], F32, tag="tp")
            nc.tensor.transpose(pt[0:cols, 0:rows], src_ap, ident[0:rows, 0:rows])
            copy_op = nc.scalar.copy if copy_engine is None else nc.vector.tensor_copy
            copy_op(out=sb_tile[0:cols, 0:rows], in_=pt[0:cols, 0:rows])

        wqT = cp.tile([D, D], F32, tag="wqT")
        transpose_to_sbuf(wq_sb[:], D, D, wqT)
        wkT = cp.tile([D, D], F32, tag="wkT")
        transpose_to_sbuf(wk_sb[:], D, D, wkT)
        wvT = cp.tile([D, D], F32, tag="wvT")
        transpose_to_sbuf(wv_sb[:], D, D, wvT)
        m2wvT = cp.tile([D, D], F32, tag="m2wvT")
        transpose_to_sbuf(m2wv_sb[:], D, D, m2wvT)

        # head-masked copies of wkT / wvT (rows outside head h zeroed) so the
        # per-head products contract over the full K=64 at base partition 0
        hmask = cp.tile([D, NH], F32, tag="hmask")
        dma(out=hmask, in_=hmask_d[:])
        wkT4 = cp.tile([D, NH, D], F32, tag="wkT4")
        wvT4 = cp.tile([D, NH, D], F32, tag="wvT4")
        for h in range(NH):
            nc.vector.tensor_scalar(out=wkT4[:, h, :], in0=wkT[:],
                                    scalar1=hmask[:, h:h + 1], scalar2=None,
                                    op0=ALU.mult)
            nc.vector.tensor_scalar(out=wvT4[:, h, :], in0=wvT[:],
                                    scalar1=hmask[:, h:h + 1], scalar2=None,
                                    op0=ALU.mult)
        # A_h = wq_h @ wk_h^T / 4
        a_ps = ps.tile([D, NH, D], F32, tag="acc")
        for h in range(NH):
            nc.tensor.matmul(a_ps[:, h, :], lhsT=wqT[:], rhs=wkT4[:, h, :])
        ah_sb = cp.tile([D, NH, D], F32, tag="ah_sb")
        nc.scalar.activation(out=ah_sb, in_=a_ps, func=AF.Copy, scale=1.0 / np.sqrt(DH))
        # M_hT[e,f] = (wv_h @ wo_h)^T
        m_ps = ps.tile([D, NH, D], F32, tag="acc")
        for h in range(NH):
            nc.tensor.matmul(m_ps[:, h, :], lhsT=wo_sb[:], rhs=wvT4[:, h, :])
        mhT_sb = cp.tile([D, NH, D], F32, tag="mhT_sb")
        nc.scalar.copy(out=mhT_sb, in_=m_ps)
        # Wvo2 = m2_wv @ m2_wo
        wvo_ps = ps.tile([D, D], F32, tag="acc")
        nc.tensor.matmul(wvo_ps, lhsT=m2wvT[:], rhs=m2wo_sb[:])
        wvo_sb = cp.tile([D, D], F32, tag="wvo_sb")
        nc.scalar.copy(out=wvo_sb, in_=wvo_ps)
        # MW_h = M_h @ Wvo2, then stacked vertically: mw_stack[h*64+f, e']
        mw_ps = ps.tile([D, NH, D], F32, tag="acc")
        for h in range(NH):
            nc.tensor.matmul(mw_ps[:, h, :], lhsT=mhT_sb[:, h, :], rhs=wvo_sb[:])
        mw_sb = cp.tile([D, NH, D], F32, tag="mw_sb")
        nc.scalar.copy(out=mw_sb, in_=mw_ps)
        mw_stack = cp.tile([128, 2, D], F32, tag="mw_stack")
        for h in range(NH):
            dma(out=mw_stack[(h % 2) * D:(h % 2 + 1) * D, h // 2, :],
                in_=mw_sb[:, h, :])

        # ================= static MLP over lab =====================================
        lab_sb = cp.tile([128, LAB + 1], F32, tag="lab_sb")
        dma(out=lab_sb[:, 0:1024], in_=lab_d[:, 0:1024])
        dma(out=lab_sb[:, 1024:LAB + 1], in_=lab_d[:, 1024:LAB + 1])
        labT = cp.tile([128, 16, 128], F32, tag="labT")
        for g in range(4):
            pt = ps.tile([128, 4, 128], F32, tag="grp")
            for i in range(4):
                kt = 4 * g + i
                w = 128 if kt < 15 else 37
                nc.tensor.transpose(pt[0:w, i, :], lab_sb[:, kt * 128:kt * 128 + w],
                                    ident[:])
            if g < 3:
                nc.vector.tensor_copy(out=labT[:, 4 * g:4 * g + 4, :], in_=pt[:])
            else:
                nc.vector.tensor_copy(out=labT[:, 12:15, :], in_=pt[:, 0:3, :])
                nc.vector.tensor_copy(out=labT[0:37, 15, :], in_=pt[0:37, 3, :])

        st1_ps = ps.tile([128, D], F32, tag="acc")
        for kt in range(16):
            k = 128 if kt < 15 else 37
            nc.tensor.matmul(st1_ps, lhsT=labT[0:k, kt, :], rhs=w1sb[0:k, kt, :],
                             start=(kt == 0), stop=(kt == 15))
        st1r = cp.tile([128, D], F32, tag="st1r")
        nc.scalar.activation(out=st1r, in_=st1_ps, func=AF.Relu)
        st1rT = cp.tile([D + 1, 128], F32, tag="st1rT")
        transpose_to_sbuf(st1r[:], 128, D, st1rT)
        nc.vector.memset(st1rT[D:D + 1, :], 1.0)
        st2_ps = ps.tile([128, H], F32, tag="acc")
        nc.tensor.matmul(st2_ps, lhsT=st1rT[:], rhs=w2sb[:])
        static_sb = cp.tile([128, H], F32, tag="static_sb")
        nc.scalar.activation(out=static_sb, in_=st2_ps, func=AF.Relu)

        # ================= glu encoder =============================================
        glu_sb = cp.tile([128, T * GLU], F32, tag="glu_sb")
        dma(out=glu_sb, in_=glu_d[:].rearrange("p j f -> p (j f)"))
        tf_sb = cp.tile([128, T * GLU], F32, tag="tf_sb")
        dma(out=tf_sb, in_=tf_d[:].rearrange("p j f -> p (j f)"))

        def transpose400(src, dst_tag, copy_op):
            pt = ps.tile([128, 4, 128], F32, tag="grp")
            for c in range(4):
                w = 128 if c < 3 else 16
                nc.tensor.transpose(pt[0:w, c, :], src[:, c * 128:c * 128 + w],
                                    ident[:])
            dst = cp.tile([128, 4, 128], F32, tag=dst_tag)
            copy_op(out=dst[:, 0:3, :], in_=pt[:, 0:3, :])
            copy_op(out=dst[0:16, 3, :], in_=pt[0:16, 3, :])
            return dst

        gluT = transpose400(glu_sb, "gluT", nc.scalar.copy)
        tfT = transpose400(tf_sb, "tfT", nc.vector.tensor_copy)

        patient = cp.tile([128, T, D], F32, tag="patient")
        # block-diagonal projection: chunk c of the (j,f)-major transpose covers
        # visits 8c..8c+7; one K=128 matmul against wbd projects all 8 at once
        gx_ps = psg.tile([128, T, H], F32, tag="gx")
        for c in range(3):
            nc.tensor.matmul(gx_ps[:, 8 * c:8 * c + 8, :], lhsT=gluT[:, c, :],
                             rhs=wbd_g[:], start=True, stop=False)
            nc.tensor.matmul(gx_ps[:, 8 * c:8 * c + 8, :], lhsT=tfT[:, c, :],
                             rhs=wbd_t[:], start=False, stop=True)
        nc.tensor.matmul(gx_ps[:, 24, :], lhsT=gluT[0:GLU, 3, :],
                         rhs=gw_g3[:], start=True, stop=False)
        nc.tensor.matmul(gx_ps[:, 24, :], lhsT=tfT[0:GLU, 3, :],
                         rhs=gw_t3[:], start=False, stop=True)
        gxb = cp.tile([128, T, H], F32, tag="gxb")
        nc.vector.tensor_add(gxb, gx_ps,
                             gbb[:].unsqueeze(1).broadcast_to((128, T, H)))
        nc.scalar.activation(out=patient[:, :, 0:H], in_=gxb, func=AF.Tanh)
        # gate = sigmoid(gx . glu_gate); patient[:, :, :H] *= gate
        gm = cp.tile([128, T, H], F32, tag="gm")
        nc.vector.tensor_mul(gm, patient[:, :, 0:H],
                             ggb[:].unsqueeze(1).broadcast_to((128, T, H)))
        gs = cp.tile([128, T], F32, tag="gs")
        nc.vector.tensor_reduce(out=gs, in_=gm, axis=AX.X, op=ALU.add)
        gsg = cp.tile([128, T], F32, tag="gsg")
        nc.scalar.activation(out=gsg, in_=gs, func=AF.Sigmoid)
        nc.vector.tensor_mul(patient[:, :, 0:H], patient[:, :, 0:H],
                             gsg[:].unsqueeze(2).broadcast_to((128, T, H)))
        # static broadcast into patient[:, :, H:D]
        nc.vector.tensor_copy(out=patient[:, :, H:D],
                              in_=static_sb[:].unsqueeze(1).broadcast_to((128, T, H)))

        # ================= med first-visit encoder =================================
        med0 = cp.tile([128, MED], F32, tag="med0")
        dma(out=med0, in_=med_d[:, 0, :])
        mb = cp.tile([128, MED + 1], F32, tag="mb")
        nc.vector.tensor_scalar(out=mb[:, 0:MED], in0=med0, scalar1=0.9,
                                scalar2=None, op0=ALU.is_gt)
        nc.vector.memset(mb[:, MED:MED + 1], 1.0)  # ones column (bias fold)
        mbTa = cp.tile([128, 128], F32, tag="mbTa")
        transpose_to_sbuf(mb[:, 0:128], 128, 128, mbTa, copy_engine=nc.vector)
        mbTb = cp.tile([18, 128], F32, tag="mbTb")
        transpose_to_sbuf(mb[:, 128:146], 128, 18, mbTb, copy_engine=nc.vector)
        x0_ps = ps.tile([128, D], F32, tag="acc")
        nc.tensor.matmul(x0_ps, lhsT=mbTa[:], rhs=mwsb[:], start=True, stop=False)
        nc.tensor.matmul(x0_ps, lhsT=mbTb[:], rhs=mw2sb[:], start=False, stop=True)
        x0 = cp.tile([128, D], F32, tag="x0")
        nc.vector.tensor_copy(out=x0, in_=x0_ps)
        scr = cp.tile([128, D], F32, tag="scr")
        nc.vector.tensor_mul(scr, x0, mgb)
        g0 = cp.tile([128, 1], F32, tag="g0")
        nc.vector.tensor_reduce(out=g0, in_=scr, axis=AX.X, op=ALU.add)
        sg0 = cp.tile([128, 1], F32, tag="sg0")
        nc.scalar.activation(out=sg0, in_=g0, func=AF.Sigmoid)
        mr0 = cp.tile([128, D], F32, tag="mr0")
        nc.vector.tensor_scalar(out=mr0, in0=x0, scalar1=sg0[:, 0:1], scalar2=None,
                                op0=ALU.mult)
        mr0T = cp.tile([D, 128], F32, tag="mr0T")
        transpose_to_sbuf(mr0[:], 128, D, mr0T, copy_engine=nc.vector)

        # ================= one-query attention =====================================
        u_ps = ps.tile([128, NH, D], F32, tag="acc")
        for h in range(NH):
            nc.tensor.matmul(u_ps[:, h, :], lhsT=mr0T[:], rhs=ah_sb[:, h, :])
        u_sb = cp.tile([128, NH, D], F32, tag="u_sb")
        nc.vector.tensor_copy(out=u_sb, in_=u_ps)

        sprod = cp.tile([128, T, NH, D], F32, tag="bigscratch")
        nc.vector.tensor_mul(sprod,
                             patient[:].unsqueeze(2).broadcast_to((128, T, NH, D)),
                             u_sb[:].unsqueeze(1).broadcast_to((128, T, NH, D)))
        s_sb = cp.tile([128, T, NH], F32, tag="s_sb")
        nc.vector.tensor_reduce(out=s_sb, in_=sprod, axis=AX.X, op=ALU.add)
        es = cp.tile([128, T, NH], F32, tag="es")
        nc.scalar.activation(out=es, in_=s_sb, func=AF.Exp)
        den = cp.tile([128, NH], F32, tag="den")
        nc.vector.tensor_reduce(out=den, in_=es.rearrange("p j h -> p h j"),
                                axis=AX.X, op=ALU.add)
        rden = cp.tile([128, NH], F32, tag="rden")
        nc.vector.reciprocal(out=rden, in_=den)
        attn = cp.tile([128, T, NH], F32, tag="attn")
        nc.vector.tensor_mul(attn, es, rden[:].unsqueeze(1).broadcast_to((128, T, NH)))

        wprod = cp.tile([128, NH, T, D], F32, tag="bigscratch2")
        nc.vector.tensor_mul(
            wprod,
            attn.rearrange("p j h -> p h j").unsqueeze(3).broadcast_to((128, NH, T, D)),
            patient[:].unsqueeze(1).broadcast_to((128, NH, T, D)))
        w_sb = cp.tile([128, NH, D], F32, tag="w_sb")
        nc.vector.tensor_reduce(out=w_sb, in_=wprod.rearrange("p h j f -> p h f j"),
                                axis=AX.X, op=ALU.add)

        wT = cp.tile([128, 2, 128], F32, tag="wT")
        wflat = w_sb.rearrange("p h f -> p (h f)")
        for c in range(2):
            pt = ps.tile([128, 128], F32, tag="tp")
            nc.tensor.transpose(pt[:], wflat[:, c * 128:(c + 1) * 128], ident[:])
            nc.vector.tensor_copy(out=wT[:, c, :], in_=pt[:])

        r_ps = ps.tile([128, D], F32, tag="acc")
        for c in range(2):
            nc.tensor.matmul(r_ps, lhsT=wT[:, c, :], rhs=mw_stack[:, c, :],
                             start=(c == 0), stop=(c == 1))
        rr = cp.tile([128, D], F32, tag="rr")
        nc.scalar.activation(out=rr, in_=r_ps, func=AF.Relu)
        rrT = cp.tile([D + 1, 128], F32, tag="rrT")
        transpose_to_sbuf(rr[:], 128, D, rrT, copy_engine=nc.vector)
        nc.vector.memset(rrT[D:D + 1, :], 1.0)

        # ================= final MLP ===============================================
        hid = cp.tile([128, HID + 1], F32, tag="hid")
        for o, n in [(0, 512), (512, 512), (1024, 136)]:
            h_ps = ps.tile([128, 512], F32, tag="acc")
            nc.tensor.matmul(h_ps[:, 0:n], lhsT=rrT[:], rhs=w1s_sb[:, o:o + n])
            nc.scalar.activation(out=hid[:, o:o + n], in_=h_ps[:, 0:n], func=AF.Relu)
        nc.vector.memset(hid[:, HID:HID + 1], 1.0)  # ones column (bias fold)
        hidT = cp.tile([128, 10, 128], F32, tag="hidT")
        for g in range(3):
            n_in_g = 4 if g < 2 else 2
            pt = ps.tile([128, 4, 128], F32, tag="grp")
            for i in range(n_in_g):
                kt = 4 * g + i
                w = 128 if kt < 9 else 9
                nc.tensor.transpose(pt[0:w, i, :], hid[:, kt * 128:kt * 128 + w],
                                    ident[:])
            if g < 2:
                nc.vector.tensor_copy(out=hidT[:, 4 * g:4 * g + 4, :], in_=pt[:])
            else:
                nc.vector.tensor_copy(out=hidT[:, 8:9, :], in_=pt[:, 0:1, :])
                nc.vector.tensor_copy(out=hidT[0:9, 9, :], in_=pt[0:9, 1, :])

        out_ps = ps.tile([128, MED], F32, tag="acc")
        for kt in range(10):
            k = 128 if kt < 9 else 9
            nc.tensor.matmul(out_ps, lhsT=hidT[0:k, kt, :], rhs=ow2sb[0:k, kt, :],
                             start=(kt == 0), stop=(kt == 9))
        out_sb = cp.tile([128, MED], F32, tag="out_sb")
        nc.vector.tensor_copy(out=out_sb, in_=out_ps)
        dma(out=out_d[:], in_=out_sb)

    if split_waits:
        split_multi_waits(nc)
    return nc


_CACHED_NC = None


def make_in_maps(inputs):
    f = lambda x: np.ascontiguousarray(np.asarray(x, dtype=np.float32))
    # out_w1 blocks [145, 64, 1160] -> 8 zero-padded shards of 19 blocks
    w1blocks = f(inputs["out_w1"]).reshape(MED, D, HID)
    shards = np.zeros((NC_CORES, MBLK, D, HID), np.float32)
    flat = np.zeros((NC_CORES * MBLK, D, HID), np.float32)
    flat[:MED] = w1blocks
    shards[:] = flat.reshape(NC_CORES, MBLK, D, HID)

    # host-side bias folding: append bias rows to weights / ones column to lab
    # (pure input marshalling; all arithmetic stays on device)
    cat = np.concatenate
    rep = {
        "sll_w1": cat([f(inputs["sll_w1"]), f(inputs["sll_b1"]).reshape(1, D)], 0),
        "sll_w2": cat([f(inputs["sll_w2"]), f(inputs["sll_b2"]).reshape(1, H)], 0),
        "glu_w": f(inputs["glu_w"]), "glu_b": f(inputs["glu_b"]).reshape(1, H),
        "glu_gate": f(inputs["glu_gate"]).reshape(1, H),
        "med_w": cat([f(inputs["med_w"]), f(inputs["med_b"]).reshape(1, D)], 0),
        "med_gate": f(inputs["med_gate"]).reshape(1, D),
        "m1_wq": f(inputs["m1_wq"]), "m1_wk": f(inputs["m1_wk"]),
        "m1_wv": f(inputs["m1_wv"]), "m1_wo": f(inputs["m1_wo"]),
        "m2_wv": f(inputs["m2_wv"]), "m2_wo": f(inputs["m2_wo"]),
        "out_b1": f(inputs["out_b1"]).reshape(1, HID),
        "out_w2": cat([f(inputs["out_w2"]), f(inputs["out_b2"]).reshape(1, MED)], 0),
    }
    lab = cat([f(inputs["lab"]), np.ones((B, 1), np.float32)], 1)
    glu, tf, med = f(inputs["glu"]), f(inputs["time_feat"]), f(inputs["med"])

    in_maps = []
    for c in range(NC_CORES):
        sl = slice(c * BC, (c + 1) * BC)
        in_maps.append({
            "lab": lab[sl], "glu": glu[sl], "tf": tf[sl], "med": med[sl],
            "w1shard": shards[c], **rep,
        })
    return in_maps


def kernel(**inputs):
    global _CACHED_NC
    if _CACHED_NC is None:
        _CACHED_NC = build_bass()
    nc = _CACHED_NC
    in_maps = make_in_maps(inputs)
    res = run_bass_kernel_spmd(nc, in_maps, core_ids=list(range(NC_CORES)))
    return np.concatenate([res.results[c]["out"] for c in range(NC_CORES)], axis=0)


if __name__ == "__main__":
    import reference
    inp = reference.setup_inputs()
    out = kernel(**{k: np.asarray(v) for k, v in inp.items()})
    print("kernel output", out.shape, out.dtype)



# revision 8
# speedup vs baseline: 1.0931x; 1.0931x over previous
"""Trainium2 Bass kernel for the MERITS_T patient model (B=1024 data-parallel over 8 cores).

Mathematical simplification of the reference (verified to ~7e-7 rel err in f32,
~4.3e-3 in bf16 against the jax reference; tolerance is 2e-2):
  - E_de = _mha(drug_mem, e0, e0) softmaxes over a single key, so its output is
    e0 @ m2_wv @ m2_wo broadcast over all 145 query rows -> the three GATs, the
    graph MHA and drug_mem never reach the output (dead code).
  - e0 = E_en[:, 0] only needs query row 0 of the m1 attention, i.e. only the
    first visit of `med`.
  - patient_j = [glu_rep_j ; static]: the static half is visit-independent, so
    it cancels in the softmax over visits and contributes static @ SMW to the
    output (SMW = sum_h (wv_h wo_h m2_wv m2_wo)[32:, :]); only the 32 glu dims
    participate in scores and the attention-weighted sum.
  - final reshape tiles r 145x, so relu(final) @ out_w1 = relu(r) @ sum_m
    out_w1[m]. The 43MB sum over m is column-sharded 8 ways: each core reduces
    its own 145-column slice fully on-device and a small bf16 AllGather
    (18.5KB/core) assembles the full [64, 1160] on every core.

All arithmetic runs on device (bf16 compute, f32 PSUM accumulation); the host
only marshals layouts (transpose / pad / concat / replicate / constant fill).
"""

import numpy as np
import ml_dtypes

import concourse.bass as bass
import concourse.mybir as mybir
from concourse.bass_utils import run_bass_kernel_spmd
from concourse.tile import TileContext

F32 = mybir.dt.float32
BF16 = mybir.dt.bfloat16
AF = mybir.ActivationFunctionType
ALU = mybir.AluOpType
AX = mybir.AxisListType


def split_multi_waits(nc):
    """The walrus on this image encodes at most ONE sync wait per TPB
    instruction ("Too many sync wait commands" otherwise). Hoist excess waits
    onto standalone InstEventSemaphore ops on the same engine, immediately
    before the instruction - the same mechanism Tile's barriers use."""
    wid = 0
    for f in nc.m.functions:
        for bb in f.blocks:
            out = []
            for ins in bb.instructions:
                si = ins.sync_info
                if si is not None and si.on_wait and len(si.on_wait) > 1:
                    waits = list(si.on_wait)
                    for w in waits[:-1]:
                        wid += 1
                        out.append(mybir.InstEventSemaphore(
                            name=f"Wsplit-{wid}", engine=ins.engine,
                            ins=[], outs=[],
                            sync_info=mybir.SyncInfo(on_wait=[w], on_update=[])))
                    si.on_wait = waits[-1:]
                out.append(ins)
            bb.instructions = out
    return wid


B, T, MED, LAB, GLU, D, H = 1024, 25, 145, 1956, 16, 64, 32
NC_CORES = 8
BC = B // NC_CORES          # 128 patients per core
NH, DH = 4, 16
HID = MED * D // 8          # 1160
CW = HID // NC_CORES        # 145 W1s columns per core

# blob column map (f32 [128, BK]; one on-device bf16 cast, then sliced)
C_WBDG, C_WBDT = 0, 256          # block-diag glu/tf weights    [r0:128]
C_GG, C_GB, C_MG = 512, 544, 576  # glu_gate/glu_b/med_gate rep [r0:128]
C_MWA, C_B1T = 640, 704          # med_w rows 0:128; b1T        [r0:128]
C_MWB = 714                      # med_w rows 128:146           [r0:18]
C_B2 = 778                       # out_b2 row                   [r0:1]
C_WOT, C_M2WVT, C_M2WO = 923, 987, 1051   # [r0:64]
C_SLW2 = 1115                    # sll_w2 + bias row            [r0:65]
C_WQT, C_WKT, C_WVT = 1147, 1403, 1531    # per-head q/k/v^T     [r0:16]
C_GW3G, C_GW3T = 1787, 1819      # visit-24 glu/tf weights      [r0:16]
BK = 1851


def build_bass(split_waits=True):
    nc = bass.Bass()

    def inp(name, shape):
        return nc.dram_tensor(name, list(shape), F32, kind="ExternalInput")

    # ---- per-core inputs (host-marshalled layouts) ----
    w1c_d = inp("w1c", (D, MED, CW))       # out_w1 column slice, d-major
    labt_d = inp("labt", (BC, 16, 128))    # lab^T k-tiles (+ones col folded)
    slwt_d = inp("slwt", (BC, 16, D))      # sll_w1 k-tiles (+bias row folded)
    glut_d = inp("glut", (BC, 4, BC))      # glu (j,f)-major transpose
    tft_d = inp("tft", (BC, 4, BC))        # time_feat ditto
    medt_d = inp("medt", (MED + 1, BC))    # med visit-0 transposed + ones row
    w2t_d = inp("w2t", (BC, 10, MED))      # out_w2 k-tiles (1160 rows 0-padded)
    blob_d = inp("blob", (BC, BK))         # packed small weights

    identity = nc.inline_tensor(np.eye(128, dtype=ml_dtypes.bfloat16),
                                name="ident128")

    # collective buffers (DRAM); AllGather assembles full W1s on every core
    cc_in = nc.dram_tensor("cc_in", [D, CW], BF16)
    cc_out = nc.dram_tensor("cc_out", [NC_CORES, D, CW], BF16,
                            addr_space="Shared")
    out_d = nc.dram_tensor("out", [BC, MED], F32, kind="ExternalOutput")

    with TileContext(nc) as tc, \
            tc.tile_pool(name="consts", bufs=1) as cp, \
            tc.tile_pool(name="ps", bufs=3, space="PSUM") as ps, \
            tc.tile_pool(name="psg", bufs=1, space="PSUM") as psg, \
            tc.tile_pool(name="pst1", bufs=1, space="PSUM") as pst1, \
            tc.tile_pool(name="pout", bufs=1, space="PSUM") as pout:

        # ================= DMA issue (order = queue execution order) ========
        # Pool/gpsimd queue: the big w1 column slice, cast f32->bf16 in-flight
        w1c_b = cp.tile([D, MED, CW], BF16, tag="w1c_b")
        nc.gpsimd.dma_start(out=w1c_b[:, 0:73, :], in_=w1c_d[:, 0:73, :])
        nc.gpsimd.dma_start(out=w1c_b[:, 73:145, :], in_=w1c_d[:, 73:145, :])

        # SP queue: blob first (unblocks all weight prep), then big f32 tiles
        blob_f = cp.tile([BC, BK], F32, tag="blob_f")
        nc.sync.dma_start(out=blob_f, in_=blob_d[:])
        labt_f = cp.tile([BC, 16, 128], F32, tag="labt_f")
        nc.sync.dma_start(out=labt_f, in_=labt_d[:])
        slwt_f = cp.tile([BC, 16, D], F32, tag="slwt_f")
        nc.sync.dma_start(out=slwt_f, in_=slwt_d[:])
        w2t_f = cp.tile([BC, 10, MED], F32, tag="w2t_f")
        nc.sync.dma_start(out=w2t_f, in_=w2t_d[:])

        # Act queue: small fast tensors feeding the longest compute chain
        glut_f = cp.tile([BC, 4, BC], F32, tag="glut_f")
        nc.scalar.dma_start(out=glut_f, in_=glut_d[:])
        tft_f = cp.tile([BC, 4, BC], F32, tag="tft_f")
        nc.scalar.dma_start(out=tft_f, in_=tft_d[:])
        med_fa = cp.tile([BC, BC], F32, tag="med_fa")
        nc.scalar.dma_start(out=med_fa, in_=medt_d[0:128, :])
        med_fb = cp.tile([18, BC], F32, tag="med_fb")
        nc.scalar.dma_start(out=med_fb, in_=medt_d[128:146, :])
        ident = cp.tile([128, 128], BF16, tag="ident")
        nc.scalar.dma_start(out=ident, in_=identity[:])

        # ================= bf16 casts (Act engine, readiness order) =========
        glut_b = cp.tile([BC, 4, BC], BF16, tag="glut_b")
        nc.scalar.copy(out=glut_b, in_=glut_f)
        tft_b = cp.tile([BC, 4, BC], BF16, tag="tft_b")
        nc.scalar.copy(out=tft_b, in_=tft_f)
        blob_b = cp.tile([BC, BK], BF16, tag="blob_b")
        nc.scalar.copy(out=blob_b, in_=blob_f)
        labt_b = cp.tile([BC, 16, 128], BF16, tag="labt_b")
        nc.scalar.copy(out=labt_b, in_=labt_f)
        slwt_b = cp.tile([BC, 16, D], BF16, tag="slwt_b")
        nc.scalar.copy(out=slwt_b, in_=slwt_f)

        # ================= med multi-hot (DVE) ==============================
        mb_a = cp.tile([BC, BC], BF16, tag="mb_a")
        nc.vector.tensor_scalar(out=mb_a, in0=med_fa, scalar1=0.9,
                                scalar2=None, op0=ALU.is_gt)
        mb_b = cp.tile([18, BC], BF16, tag="mb_b")
        nc.vector.tensor_scalar(out=mb_b, in0=med_fb, scalar1=0.9,
                                scalar2=None, op0=ALU.is_gt)

        # ================= glu encoder matmuls (PE block-diagonal) ==========
        gx_ps = psg.tile([BC, T, H], F32, tag="gx")
        for c in range(3):
            nc.tensor.matmul(gx_ps[:, 8 * c:8 * c + 8, :],
                             lhsT=glut_b[:, c, :],
                             rhs=blob_b[:, C_WBDG:C_WBDG + 256],
                             start=True, stop=False)
            nc.tensor.matmul(gx_ps[:, 8 * c:8 * c + 8, :],
                             lhsT=tft_b[:, c, :],
                             rhs=blob_b[:, C_WBDT:C_WBDT + 256],
                             start=False, stop=True)
        nc.tensor.matmul(gx_ps[:, 24, :], lhsT=glut_b[0:16, 3, :],
                         rhs=blob_b[0:16, C_GW3G:C_GW3G + 32],
                         start=True, stop=False)
        nc.tensor.matmul(gx_ps[:, 24, :], lhsT=tft_b[0:16, 3, :],
                         rhs=blob_b[0:16, C_GW3T:C_GW3T + 32],
                         start=False, stop=True)

        # med x0 = multihot @ med_w + med_b (bias row folded via ones row)
        x0_ps = ps.tile([BC, D], F32, tag="acc")
        nc.tensor.matmul(x0_ps, lhsT=mb_a,
                         rhs=blob_b[:, C_MWA:C_MWA + D], start=True, stop=False)
        nc.tensor.matmul(x0_ps, lhsT=mb_b,
                         rhs=blob_b[0:18, C_MWB:C_MWB + D], start=False, stop=True)

        # ================= weight prep (PE + gpsimd copies) =================
        # A_h[:, :32] = wq_h wk_h^T / 4 (glu columns only), stacked [64,(h,32)]
        a_ps = ps.tile([D, NH, H], F32, tag="acc")
        for h in range(NH):
            nc.tensor.matmul(a_ps[:, h, :],
                             lhsT=blob_b[0:16, C_WQT + 64 * h:C_WQT + 64 * h + 64],
                             rhs=blob_b[0:16, C_WKT + 32 * h:C_WKT + 32 * h + 32])
        # Wvo2 = m2_wv @ m2_wo
        wvo_ps = ps.tile([D, D], F32, tag="acc")
        nc.tensor.matmul(wvo_ps, lhsT=blob_b[0:D, C_M2WVT:C_M2WVT + D],
                         rhs=blob_b[0:D, C_M2WO:C_M2WO + D])

        a_sb = cp.tile([D, NH, H], BF16, tag="a_sb")
        nc.scalar.activation(out=a_sb, in_=a_ps, func=AF.Copy, scale=0.25)
        wvo_sb = cp.tile([D, D], BF16, tag="wvo_sb")
        nc.scalar.copy(out=wvo_sb, in_=wvo_ps)

        # t_h = wo_h @ Wvo2  [16,(h),64]
        t_ps = ps.tile([16, NH, D], F32, tag="acc")
        for h in range(NH):
            nc.tensor.matmul(t_ps[:, h, :],
                             lhsT=blob_b[0:D, C_WOT + 16 * h:C_WOT + 16 * h + 16],
                             rhs=wvo_sb)
        t_sb = cp.tile([16, NH, D], BF16, tag="t_sb")
        nc.scalar.copy(out=t_sb, in_=t_ps)
        # mwT[e, h, f] = MW_h[f, e] = (wv_h t_h)[f, e], computed transposed
        mwt_ps = ps.tile([D, NH, D], F32, tag="acc")
        for h in range(NH):
            nc.tensor.matmul(mwt_ps[:, h, :], lhsT=t_sb[:, h, :],
                             rhs=blob_b[0:16, C_WVT + 64 * h:C_WVT + 64 * h + 64])
        mwtg_sb = cp.tile([D, NH, H], BF16, tag="mwtg_sb")
        nc.scalar.copy(out=mwtg_sb, in_=mwt_ps[:, :, 0:H])
        mwth_sb = cp.tile([D, NH, H], BF16, tag="mwth_sb")
        nc.scalar.copy(out=mwth_sb, in_=mwt_ps[:, :, H:D])
        # MWg_stack[(h,f<32), e] via one PE transpose of mwT[:, :, :32]
        mwg_ps = ps.tile([128, D], BF16, tag="acc")
        nc.tensor.transpose(mwg_ps, mwtg_sb[:].rearrange("p h f -> p (h f)"),
                            ident[0:D, 0:D])
        mwg_sb = cp.tile([128, D], BF16, tag="mwg_sb")
        nc.scalar.copy(out=mwg_sb, in_=mwg_ps)
        # SMW = sum_h MW_h[32:, :]: sum mwT halves then transpose
        smwt = cp.tile([D, H], BF16, tag="smwt")
        nc.gpsimd.tensor_tensor(out=smwt, in0=mwth_sb[:, 0, :],
                                in1=mwth_sb[:, 1, :], op=ALU.add)
        nc.gpsimd.tensor_tensor(out=smwt, in0=smwt, in1=mwth_sb[:, 2, :],
                                op=ALU.add)
        nc.gpsimd.tensor_tensor(out=smwt, in0=smwt, in1=mwth_sb[:, 3, :],
                                op=ALU.add)
        smw_ps = ps.tile([H, D], BF16, tag="acc")
        nc.tensor.transpose(smw_ps, smwt[:], ident[0:D, 0:D])
        smw_sb = cp.tile([H, D], BF16, tag="smw_sb")
        nc.scalar.copy(out=smw_sb, in_=smw_ps)
        st1rt = cp.tile([D + 1, BC], BF16, tag="st1rt")
        nc.gpsimd.memset(st1rt[D:D + 1, :], 1.0)

        # ================= med gating -> mr0 -> u (critical chain) ==========
        scr = cp.tile([BC, D], BF16, tag="scr")
        nc.vector.tensor_mul(scr, x0_ps, blob_b[:, C_MG:C_MG + D])
        g0 = cp.tile([BC, 1], F32, tag="g0")
        nc.vector.tensor_reduce(out=g0, in_=scr, axis=AX.X, op=ALU.add)
        sg0 = cp.tile([BC, 1], F32, tag="sg0")
        nc.scalar.activation(out=sg0, in_=g0, func=AF.Sigmoid)
        mr0 = cp.tile([BC, D], BF16, tag="mr0")
        nc.vector.tensor_scalar(out=mr0, in0=x0_ps, scalar1=sg0[:, 0:1],
                                scalar2=None, op0=ALU.mult)
        mr0t_ps = ps.tile([D, BC], BF16, tag="acc")
        nc.tensor.transpose(mr0t_ps, mr0[:], ident[:])
        mr0t = cp.tile([D, BC], BF16, tag="mr0t")
        nc.vector.tensor_copy(out=mr0t, in_=mr0t_ps)
        u_ps = ps.tile([BC, NH, H], F32, tag="acc")
        nc.tensor.matmul(u_ps[:].rearrange("p h e -> p (h e)"), lhsT=mr0t,
                         rhs=a_sb[:].rearrange("p h e -> p (h e)"))
        u_sb = cp.tile([BC, NH, H], BF16, tag="u_sb")
        nc.vector.tensor_copy(out=u_sb, in_=u_ps)

        # ================= glu encoder tail (DVE/Act) =======================
        gxb = cp.tile([BC, T, H], BF16, tag="gxb")
        nc.vector.tensor_add(gxb, gx_ps,
                             blob_b[:, C_GB:C_GB + H].unsqueeze(1)
                             .broadcast_to((BC, T, H)))
        grep = cp.tile([BC, T, H], BF16, tag="grep")
        nc.scalar.activation(out=grep, in_=gxb, func=AF.Tanh)
        gm = cp.tile([BC, T, H], BF16, tag="gm")
        nc.vector.tensor_mul(gm, grep,
                             blob_b[:, C_GG:C_GG + H].unsqueeze(1)
                             .broadcast_to((BC, T, H)))
        gs = cp.tile([BC, T], F32, tag="gs")
        nc.vector.tensor_reduce(out=gs, in_=gm, axis=AX.X, op=ALU.add)
        gsg = cp.tile([BC, T], BF16, tag="gsg")
        nc.scalar.activation(out=gsg, in_=gs, func=AF.Sigmoid)
        nc.vector.tensor_mul(grep, grep,
                             gsg[:].unsqueeze(2).broadcast_to((BC, T, H)))

        # ================= one-query attention (glu dims only) ==============
        sprod = cp.tile([BC, NH, T, H], BF16, tag="sprod")
        nc.vector.tensor_mul(
            sprod,
            grep[:].unsqueeze(1).broadcast_to((BC, NH, T, H)),
            u_sb[:].unsqueeze(2).broadcast_to((BC, NH, T, H)))
        # halving-tree reduce over f (keeps fast bf16 DVE throughput)
        for wdt in (16, 8, 4, 2, 1):
            nc.vector.tensor_add(sprod[:, :, :, 0:wdt], sprod[:, :, :, 0:wdt],
                                 sprod[:, :, :, wdt:2 * wdt])
        es = cp.tile([BC, NH, T], BF16, tag="es")
        nc.scalar.activation(out=es, in_=sprod[:, :, :, 0], func=AF.Exp)
        den = cp.tile([BC, NH], F32, tag="den")
        nc.vector.tensor_reduce(out=den, in_=es, axis=AX.X, op=ALU.add)
        rden = cp.tile([BC, NH], F32, tag="rden")
        nc.vector.reciprocal(out=rden, in_=den)
        attn = cp.tile([BC, NH, T], BF16, tag="attn")
        nc.vector.tensor_mul(attn, es,
                             rden[:].unsqueeze(2).broadcast_to((BC, NH, T)))
        # weighted sum over visits, f-major so innermost stays packed
        grept = cp.tile([BC, H, T], BF16, tag="grept")
        nc.vector.tensor_copy(out=grept,
                              in_=grep[:].rearrange("p j f -> p f j"))
        wprod = cp.tile([BC, NH, H, T], BF16, tag="wprod")
        nc.vector.tensor_mul(
            wprod,
            attn[:].unsqueeze(2).broadcast_to((BC, NH, H, T)),
            grept[:].unsqueeze(1).broadcast_to((BC, NH, H, T)))
        nc.vector.tensor_add(wprod[:, :, :, 0:9], wprod[:, :, :, 0:9],
                             wprod[:, :, :, 16:25])
        for wdt in (8, 4, 2, 1):
            nc.vector.tensor_add(wprod[:, :, :, 0:wdt], wprod[:, :, :, 0:wdt],
                                 wprod[:, :, :, wdt:2 * wdt])
        wfin = cp.tile([BC, NH, H], BF16, tag="wfin")
        nc.vector.tensor_copy(out=wfin[:].unsqueeze(3), in_=wprod[:, :, :, 0:1])

        # ================= W1s column reduce + AllGather ====================
        def tree(lo, n):  # reduce w1c_b[:, lo:lo+n] into w1c_b[:, lo]
            while n > 1:
                half, odd = n // 2, n % 2
                if odd:
                    nc.vector.tensor_add(w1c_b[:, lo:lo + 1, :],
                                         w1c_b[:, lo:lo + 1, :],
                                         w1c_b[:, lo + n - 1:lo + n, :])
                    n -= 1
                nc.vector.tensor_add(w1c_b[:, lo:lo + half, :],
                                     w1c_b[:, lo:lo + half, :],
                                     w1c_b[:, lo + half:lo + 2 * half, :])
                n = half

        tree(0, 73)
        tree(73, 72)
        w1sc = cp.tile([D, CW], BF16, tag="w1sc")
        nc.vector.tensor_add(w1sc[:].unsqueeze(1), w1c_b[:, 0:1, :],
                             w1c_b[:, 73:74, :])
        nc.scalar.dma_start(out=cc_in[:], in_=w1sc[:])
        nc.gpsimd.collective_compute(
            "AllGather", ALU.bypass, replica_groups=[list(range(NC_CORES))],
            ins=[cc_in[:]], outs=[cc_out[:]])

        # ================= static MLP (transposed; no lab transpose) ========
        st1_ps = pst1.tile([D, BC], F32, tag="st1")
        for t in range(16):
            nc.tensor.matmul(st1_ps, lhsT=slwt_b[:, t, :], rhs=labt_b[:, t, :],
                             start=(t == 0), stop=(t == 15))
        nc.scalar.activation(out=st1rt[0:D, :], in_=st1_ps, func=AF.Relu)
        stat_ps = ps.tile([H, BC], F32, tag="acc")
        nc.tensor.matmul(stat_ps, lhsT=blob_b[0:D + 1, C_SLW2:C_SLW2 + H],
                         rhs=st1rt)
        statt = cp.tile([H, BC], BF16, tag="statt")
        nc.scalar.activation(out=statt, in_=stat_ps, func=AF.Relu)

        # ================= r = attention out + static part ==================
        wgt_ps = ps.tile([128, BC], BF16, tag="acc")
        nc.tensor.transpose(wgt_ps, wfin[:].rearrange("p h f -> p (h f)"),
                            ident[:])
        wgt_sb = cp.tile([128, BC], BF16, tag="wgt_sb")
        nc.vector.tensor_copy(out=wgt_sb, in_=wgt_ps)
        r_ps = ps.tile([BC, D], F32, tag="acc")
        nc.tensor.matmul(r_ps, lhsT=statt, rhs=smw_sb, start=True, stop=False)
        nc.tensor.matmul(r_ps, lhsT=wgt_sb, rhs=mwg_sb, start=False, stop=True)
        rr = cp.tile([BC, D], BF16, tag="rr")
        nc.scalar.activation(out=rr, in_=r_ps, func=AF.Relu)
        rrt_ps = ps.tile([D, BC], BF16, tag="acc")
        nc.tensor.transpose(rrt_ps, rr[:], ident[:])
        rrt = cp.tile([D, BC], BF16, tag="rrt")
        nc.vector.tensor_copy(out=rrt, in_=rrt_ps)

        # w2 cast late on Act queue (only needed after the AllGather)
        w2t_b = cp.tile([BC, 10, MED], BF16, tag="w2t_b")
        nc.scalar.copy(out=w2t_b, in_=w2t_f)
        ones_sb = cp.tile([1, BC], BF16, tag="ones_sb")
        nc.gpsimd.memset(ones_sb, 1.0)

        # ================= final MLP (after AllGather) ======================
        w1s_sb = cp.tile([D, HID], BF16, tag="w1s_sb")
        nc.scalar.dma_start(out=w1s_sb[:].rearrange("d (c i) -> d c i", c=8),
                            in_=cc_out[:].rearrange("c d i -> d c i"))
        hidt = cp.tile([128, 10, 128], BF16, tag="hidt")
        out_ps = pout.tile([BC, MED], F32, tag="outacc")
        nc.tensor.matmul(out_ps, lhsT=ones_sb,
                         rhs=blob_b[0:1, C_B2:C_B2 + MED],
                         start=True, stop=False, skip_group_check=True)
        for t in range(10):
            w = 128 if t < 9 else 8
            h_ps = ps.tile([128, BC], F32, tag="acc")
            nc.tensor.matmul(h_ps[0:w, :],
                             lhsT=w1s_sb[:, 128 * t:128 * t + w], rhs=rrt)
            nc.scalar.activation(out=hidt[0:w, t, :], in_=h_ps[0:w, :],
                                 func=AF.Relu,
                                 bias=blob_b[0:w, C_B1T + t:C_B1T + t + 1])
            nc.tensor.matmul(out_ps, lhsT=hidt[0:w, t, :],
                             rhs=w2t_b[0:w, t, :],
                             start=False, stop=(t == 9), skip_group_check=True)
        out_sb = cp.tile([BC, MED], F32, tag="out_sb")
        nc.scalar.copy(out=out_sb, in_=out_ps)
        nc.scalar.dma_start(out=out_d[:], in_=out_sb)

    if split_waits:
        split_multi_waits(nc)
    return nc


_CACHED_NC = None


def make_in_maps(inputs):
    """Host-side input marshalling: pure layout work (transpose / pad / concat
    / replicate / constant fill) - every arithmetic op stays on device."""
    f = lambda x: np.ascontiguousarray(np.asarray(x, dtype=np.float32))
    cat = np.concatenate

    # out_w1 [9280, 1160] -> [145, 64, 1160] -> per-core d-major column slice
    w1 = f(inputs["out_w1"]).reshape(MED, D, HID)

    # lab^T k-tiles with ones column folded at row 1957
    lab = f(inputs["lab"])
    lab_ext = np.zeros((B, 2048), np.float32)
    lab_ext[:, :LAB] = lab
    lab_ext[:, LAB] = 1.0           # ones column folds sll_b1 into the matmul

    slw = np.zeros((2048, D), np.float32)
    slw[:LAB] = f(inputs["sll_w1"])
    slw[LAB] = f(inputs["sll_b1"])
    slwt = np.ascontiguousarray(slw.reshape(16, 128, D).transpose(1, 0, 2))

    glu, tf = f(inputs["glu"]), f(inputs["time_feat"])

    def jf_major(x):  # (j, f)-major transpose, padded 400 -> 512
        z = np.zeros((B, 512), np.float32)
        z[:, :T * GLU] = x.reshape(B, T * GLU)
        return z.reshape(B, 4, 128).transpose(2, 1, 0)  # [128p, 4c, B]

    glut, tft = jf_major(glu), jf_major(tf)

    med0 = f(inputs["med"])[:, 0, :]                  # [B, 145]
    medt = np.ones((MED + 1, B), np.float32)          # row 145 = 1.0
    medt[:MED] = med0.T

    w2 = np.zeros((1280, MED), np.float32)
    w2[:HID] = f(inputs["out_w2"])
    w2t = np.ascontiguousarray(w2.reshape(10, 128, MED).transpose(1, 0, 2))

    # ---- packed weight blob ----
    blob = np.zeros((BC, BK), np.float32)
    glu_w = f(inputs["glu_w"])                         # [32, 32]
    for jl in range(8):
        blob[16 * jl:16 * jl + 16,
             C_WBDG + 32 * jl:C_WBDG + 32 * jl + 32] = glu_w[:16]
        blob[16 * jl:16 * jl + 16,
             C_WBDT + 32 * jl:C_WBDT + 32 * jl + 32] = glu_w[16:]
    blob[:, C_GG:C_GG + H] = f(inputs["glu_gate"])[None, :]
    blob[:, C_GB:C_GB + H] = f(inputs["glu_b"])[None, :]
    blob[:, C_MG:C_MG + D] = f(inputs["med_gate"])[None, :]
    medw_ext = cat([f(inputs["med_w"]), f(inputs["med_b"])[None, :]], 0)
    blob[:, C_MWA:C_MWA + D] = medw_ext[:128]
    blob[0:18, C_MWB:C_MWB + D] = medw_ext[128:]
    b1 = f(inputs["out_b1"])
    for t in range(10):
        nvalid = 128 if t < 9 else 8
        blob[:nvalid, C_B1T + t] = b1[128 * t:128 * t + nvalid]
    blob[0, C_B2:C_B2 + MED] = f(inputs["out_b2"])
    wo, wv = f(inputs["m1_wo"]), f(inputs["m1_wv"])
    wq, wk = f(inputs["m1_wq"]), f(inputs["m1_wk"])
    blob[0:D, C_WOT:C_WOT + D] = wo.T                  # woT[d, (h,g)]
    blob[0:D, C_M2WVT:C_M2WVT + D] = f(inputs["m2_wv"]).T
    blob[0:D, C_M2WO:C_M2WO + D] = f(inputs["m2_wo"])
    blob[0:D, C_SLW2:C_SLW2 + H] = f(inputs["sll_w2"])
    blob[D, C_SLW2:C_SLW2 + H] = f(inputs["sll_b2"])
    for h in range(NH):
        blob[0:16, C_WQT + 64 * h:C_WQT + 64 * h + 64] = wq[:, 16 * h:16 * h + 16].T
        blob[0:16, C_WKT + 32 * h:C_WKT + 32 * h + 32] = wk[:H, 16 * h:16 * h + 16].T
        blob[0:16, C_WVT + 64 * h:C_WVT + 64 * h + 64] = wv[:, 16 * h:16 * h + 16].T
    blob[0:16, C_GW3G:C_GW3G + H] = glu_w[:16]
    blob[0:16, C_GW3T:C_GW3T + H] = glu_w[16:]

    in_maps = []
    for c in range(NC_CORES):
        sl = slice(c * BC, (c + 1) * BC)
        in_maps.append({
            "w1c": np.ascontiguousarray(
                w1[:, :, c * CW:(c + 1) * CW].transpose(1, 0, 2)),
            "labt": np.ascontiguousarray(
                lab_ext[sl].T.reshape(16, 128, BC).transpose(1, 0, 2)),
            "slwt": slwt,
            "glut": np.ascontiguousarray(glut[:, :, sl]),
            "tft": np.ascontiguousarray(tft[:, :, sl]),
            "medt": np.ascontiguousarray(medt[:, sl]),
            "w2t": w2t,
            "blob": blob,
        })
    return in_maps


def kernel(**inputs):
    global _CACHED_NC
    if _CACHED_NC is None:
        _CACHED_NC = build_bass()
    nc = _CACHED_NC
    in_maps = make_in_maps(inputs)
    res = run_bass_kernel_spmd(nc, in_maps, core_ids=list(range(NC_CORES)))
    return np.concatenate([res.results[c]["out"] for c in range(NC_CORES)], axis=0)


if __name__ == "__main__":
    import reference
    inp = reference.setup_inputs()
    out = kernel(**{k: np.asarray(v) for k, v in inp.items()})
    print("kernel output", out.shape, out.dtype)
